# revision 1
# baseline (speedup 1.0000x reference)
"""MoE top-1 routing kernel for Trainium2 (8 NeuronCores, expert-parallel).

Model (E=8, D=512, F=2048, N=4096):
    logits = x @ Wg + bg; e = argmax(logits)
    y[i] = relu(x[i] @ W1[e] + b1[e]) @ W2[e] + b2[e]

Strategy:
- Host computes the gate (f64 matmul + argmax) and routes tokens; core e gets
  only expert e's tokens (padded to the max expert count C) + expert e's
  weights, and runs a dense 2-layer MLP in fp32r (full-rate fp32 matmul mode,
  ~2e-4 scale-relative error).
- All tensors are host-packed into SBUF-native [128, *] layouts so every DMA
  moves multi-KB contiguous runs per partition (one dma_start per piece).
- DMA pieces are issued in consumption order (x chunk 0, W1 by f-groups,
  W2 by fo-groups) so matmuls start ~5us in; stage-2 accumulation is emitted
  in W2-piece arrival order, interleaved across PSUM banks.
- A short dummy-matmul burst warms the PE clock (HAM) during the DMA head.
- Tokens ride the matmul free dim in chunks of <=512 columns (PSUM bank
  limit), >=256 wide where possible (fp32r full-rate threshold).
"""

import sys

sys.path.insert(0, "/opt/trn_rl_repo")

import numpy as np

E, D, F, N_CORES = 8, 512, 2048, 8
KD, KF = D // 128, F // 128  # 4, 16
G1, G2 = KF // 4, KF // 4    # w1 f-piece count, w2 fo-piece count (4 each)

_cache: dict = {}


def _build(C: int, chunks: list[tuple[int, int]]):
    import concourse.tile as tile
    import concourse.mybir as mybir
    from concourse import bacc

    f32, f32r = mybir.dt.float32, mybir.dt.float32r
    Relu = mybir.ActivationFunctionType.Relu

    nc = bacc.Bacc("TRN2", target_bir_lowering=False, debug=False)
    # packed layouts, all [128, *]:
    #   xTi[p, chunk_off + ko*cw + c] = x_e[c0+c, 128*ko+p]
    #   w1i[p, g*2048 + ko*512 + fi]  = W1_e[128*ko+p, 512*g+fi]
    #   w2i[p, h*2048 + j*512 + d]    = W2_e[128*(4h+j)+p, d]
    #   bi[p, f] = b1_e[128f+p] (f<16);  bi[p, 16+d] = b2_e[128d+p]
    #   yTi[p, d*C + c] = y_e[c, 128d+p]
    xTi = nc.dram_tensor("xTi", [128, KD * C], f32r, kind="ExternalInput").ap()
    w1i = nc.dram_tensor("w1i", [128, KD * F], f32r, kind="ExternalInput").ap()
    w2i = nc.dram_tensor("w2i", [128, KF * D], f32r, kind="ExternalInput").ap()
    bi = nc.dram_tensor("bi", [128, KF + KD], f32, kind="ExternalInput").ap()
    yTi = nc.dram_tensor("yTi", [128, KD * C], f32, kind="ExternalOutput").ap()
    y3 = yTi.rearrange("p (d c) -> p d c", c=C)

    with tile.TileContext(nc) as tc:
        with tc.tile_pool(name="wp", bufs=1) as wp, \
             tc.tile_pool(name="hp", bufs=1) as hp, \
             tc.tile_pool(name="yp", bufs=2) as yp, \
             tc.tile_pool(name="scr", bufs=1) as scr, \
             tc.tile_pool(name="pp", bufs=3, space="PSUM") as pp:

            # --- PE warm-up: dummy matmuls during the DMA head (HAM ramp).
            # f32 runs 4 cycles/row: N=128 -> ~427ns cold each, so 9 of them
            # cover the ~3.4us HAM window while delaying real matmuls <0.5us.
            wrm = scr.tile([128, 128], f32, name="wrm")
            nc.vector.memset(wrm[:], 0.0)
            wps = pp.tile([128, 128], f32, name="wps", tag="wps", bufs=1)
            for _ in range(14):
                nc.tensor.matmul(wps[:], wrm[:], wrm[:], start=True, stop=True)

            # --- DMA issue, consumption order, single engine (sync) ---
            bis = wp.tile([128, KF + KD], f32, name="bis")
            nc.scalar.dma_start(bis[:], bi[:])

            w1t = wp.tile([128, KD * F], f32r, name="w1t")
            w2t = wp.tile([128, KF * D], f32r, name="w2t")
            xs = []
            off = 0
            for ci, (c0, c1) in enumerate(chunks):
                cw = c1 - c0
                xst = wp.tile([128, KD * cw], f32r, name=f"xs{ci}", tag=f"xs{ci}")
                xs.append((xst, off))
                off += KD * cw
            # Issue order = consumption order, serial on sync (parallel
            # multi-engine issue measured slower; aggregate is HBM-bound at
            # ~390GB/s once a few dma_starts are outstanding).
            def dma_piece(dst, src, lo, hi):
                nc.sync.dma_start(dst[:, lo:hi], src[:, lo:hi])

            xst0, o0 = xs[0]
            half = xst0.shape[-1] // 2
            nc.sync.dma_start(xst0[:, :half], xTi[:, o0:o0 + half])
            dma_piece(w1t, w1i, 0, 1024)
            nc.sync.dma_start(xst0[:, half:], xTi[:, o0 + half:o0 + xst0.shape[-1]])
            dma_piece(w1t, w1i, 1024, 2048)
            if len(xs) > 1:
                xst1, o1 = xs[1]
                nc.sync.dma_start(xst1[:], xTi[:, o1:o1 + xst1.shape[-1]])
            for g in range(1, G1):
                dma_piece(w1t, w1i, g * 2048, g * 2048 + 1024)
                dma_piece(w1t, w1i, g * 2048 + 1024, (g + 1) * 2048)
            for xst, o in xs[2:]:
                nc.sync.dma_start(xst[:], xTi[:, o:o + xst.shape[-1]])
            for h in range(G2):
                dma_piece(w2t, w2i, h * 2048, h * 2048 + 1024)
                dma_piece(w2t, w2i, h * 2048 + 1024, (h + 1) * 2048)

            # --- stage 1: h = relu(x @ W1 + b1), emitted in w1-piece order ---
            hs = {}  # (ci, f) -> tile
            for g in range(G1):
                for ci, (c0, c1) in enumerate(chunks):
                    cw = c1 - c0
                    xst = xs[ci][0]
                    for f in range(4 * g, 4 * g + 4):
                        p1 = pp.tile([128, cw], f32, name=f"p1_{ci}_{f}", tag="p1")
                        for ko in range(KD):
                            lhsT = w1t[:, g * 2048 + ko * 512 + (f % 4) * 128:
                                       g * 2048 + ko * 512 + (f % 4) * 128 + 128]
                            nc.tensor.matmul(p1[:], lhsT, xst[:, ko * cw:(ko + 1) * cw],
                                             start=(ko == 0), stop=(ko == KD - 1))
                        h = hp.tile([128, cw], f32r, name=f"h{ci}_{f}",
                                    tag=f"h{ci % 2}_{f}")
                        nc.scalar.activation(h[:], p1[:], Relu, bias=bis[:, f:f + 1])
                        hs[(ci, f)] = h

            # --- stage 2: y = h @ W2 + b2 ---
            # Early chunks run w2-piece-arrival-major (h outer) to follow the
            # DMA stream; the last chunk runs d-major (w2 fully resident by
            # then) so each d's bias-add + output DMA starts as soon as that
            # d finishes, overlapping the kernel tail.
            for ci, (c0, c1) in enumerate(chunks):
                cw = c1 - c0
                last = ci == len(chunks) - 1
                p2s = [pp.tile([128, cw], f32, name=f"p2_{ci}_{d}", tag=f"p2_{d}",
                               bufs=1) for d in range(KD)]
                ys = yp.tile([128, KD, cw], f32, name=f"ys{ci}", tag="ys")
                out_engs = [nc.gpsimd, nc.scalar, nc.gpsimd, nc.scalar]

                def s2_mm(d, fo):
                    h2, j = divmod(fo, 4)
                    lhsT = w2t[:, h2 * 2048 + j * 512 + d * 128:
                               h2 * 2048 + j * 512 + d * 128 + 128]
                    nc.tensor.matmul(p2s[d][:], lhsT, hs[(ci, fo)][:],
                                     start=(fo == 0), stop=(fo == KF - 1))

                def s2_out(d):
                    nc.vector.tensor_scalar_add(ys[:, d, :], p2s[d][:],
                                                bis[:, KF + d:KF + d + 1])
                    out_engs[d].dma_start(y3[:, d, c0:c1], ys[:, d, :])

                if last:
                    for d in range(KD):
                        for fo in range(KF):
                            s2_mm(d, fo)
                        s2_out(d)
                else:
                    for h2 in range(G2):
                        for d in range(KD):
                            for j in range(4):
                                s2_mm(d, 4 * h2 + j)
                    for d in range(KD):
                        s2_out(d)
    nc.compile()
    return nc


def _plan_chunks(C: int) -> list[tuple[int, int]]:
    n = max(1, -(-C // 512))
    base, rem = divmod(C, n)
    out, pos = [], 0
    for i in range(n):
        w = base + (1 if i < rem else 0)
        out.append((pos, pos + w))
        pos += w
    return out


def _get_nc(C: int):
    if C not in _cache:
        _cache[C] = _build(C, _plan_chunks(C))
    return _cache[C]


def _pack_inputs(x, W1, b1, W2, b2, idx, order, starts, C):
    chunks = _plan_chunks(C)
    in_maps, toks_per_core = [], []
    for e in range(E):
        toks = order[starts[e]:starts[e + 1]]
        toks_per_core.append(toks)
        xe = np.zeros((C, D), np.float32)
        xe[:len(toks)] = x[toks]
        xeT = xe.T  # [D, C]
        xTi = np.concatenate(
            [xeT[:, c0:c1].reshape(KD, 128, c1 - c0).transpose(1, 0, 2)
             .reshape(128, KD * (c1 - c0)) for c0, c1 in chunks], axis=1)
        w1p = np.concatenate(
            [W1[e][:, 512 * g:512 * (g + 1)].reshape(KD, 128, 512)
             .transpose(1, 0, 2).reshape(128, KD * 512) for g in range(G1)], axis=1)
        w2p = np.concatenate(
            [W2[e][512 * h:512 * (h + 1), :].reshape(4, 128, 512)
             .transpose(1, 0, 2).reshape(128, 4 * 512) for h in range(G2)], axis=1)
        bi = np.concatenate([b1[e].reshape(KF, 128).T,
                             b2[e].reshape(KD, 128).T], axis=1)
        in_maps.append({
            "xTi": np.ascontiguousarray(xTi),
            "w1i": np.ascontiguousarray(w1p),
            "w2i": np.ascontiguousarray(w2p),
            "bi": np.ascontiguousarray(bi),
        })
    return in_maps, toks_per_core, chunks


def kernel(x, Wg, bg, W1, b1, W2, b2):
    from concourse.bass_utils import run_bass_kernel_spmd

    x = np.asarray(x, dtype=np.float32)
    n_tok = x.shape[0]

    # host gate in f64: the mathematically-true argmax
    logits = x.astype(np.float64) @ np.asarray(Wg, np.float64) + np.asarray(bg, np.float64)
    idx = logits.argmax(1)

    counts = np.bincount(idx, minlength=E)
    order = np.argsort(idx, kind="stable")
    starts = np.zeros(E + 1, np.int64)
    starts[1:] = np.cumsum(counts)

    C = max(int(counts.max()), 256)
    C = (C + 15) // 16 * 16

    W1 = np.asarray(W1, np.float32)
    W2 = np.asarray(W2, np.float32)
    b1 = np.asarray(b1, np.float32)
    b2 = np.asarray(b2, np.float32)

    in_maps, toks_per_core, chunks = _pack_inputs(x, W1, b1, W2, b2,
                                                  idx, order, starts, C)
    nc = _get_nc(C)
    res = run_bass_kernel_spmd(nc, in_maps, core_ids=list(range(N_CORES)))

    out = np.zeros((n_tok, D), np.float32)
    for e in range(E):
        toks = toks_per_core[e]
        ye = res.results[e]["yTi"].reshape(128, KD, C).transpose(2, 1, 0) \
            .reshape(C, D)
        out[toks] = ye[:len(toks)]
    return out



# revision 2
# speedup vs baseline: 1.0368x; 1.0368x over previous
"""MoE top-1 routing kernel for Trainium2 (8 NeuronCores, expert-parallel).

Model (E=8, D=512, F=2048, N=4096):
    logits = x @ Wg + bg; e = argmax(logits)
    y[i] = relu(x[i] @ W1[e] + b1[e]) @ W2[e] + b2[e]

Strategy (v2, fp16):
- Host computes the gate (f64 matmul + argmax) and routes tokens; core e gets
  only expert e's tokens (padded to the max expert count C) + expert e's
  weights, and runs a dense 2-layer MLP with fp16 operands (weights, x, h) and
  fp32 PSUM accumulation (~3e-4 rel err vs f64 oracle, threshold 2e-2).
- fp16 halves HBM traffic vs fp32r (4.7MB/core total) and avoids the fp32r
  LOW_HIGH double-pass on the PE.
- All tensors are host-packed into SBUF-native [128, *] layouts; DMA pieces
  are issued on the sync engine in consumption order, with the first pieces
  (bias, w1 ko0 slice, x chunk-A ko0 slice) kept small so real matmuls start
  as soon as the engines boot (~7.5us).
- A short warm-up burst (6 thin fp16 matmuls on a memset tile) keeps the PE
  busy from engine-boot until real data lands, so the HAM clock-gate ramps to
  2.4GHz early; the burst is thin enough not to delay real work.
- Stage-1 waves are emitted ko0-first (4 matmuls needing only the first DMA
  pieces), then f-major ko1-3 + ReLU activation per f-tile; 4 PSUM banks.
- Stage-2 is d-major with 2 double-buffered PSUM tags; each d's bias-add
  (vector) and output DMA (scalar, fp16) streams while the PE continues.
- Tile/tag/queue counts are kept minimal: the TileContext teardown (semaphore
  resets) lands inside the measured window, so fewer semaphores = faster.
"""

import sys

sys.path.insert(0, "/opt/trn_rl_repo")

import numpy as np

E, D, F, N_CORES = 8, 512, 2048, 8
KD, KF = D // 128, F // 128  # 4, 16
G1, G2 = KF // 4, KF // 4    # w1 f-group count, w2 fo-group count (4 each)

_cache: dict = {}


def _build(C: int, chunks: list[tuple[int, int]]):
    import concourse.tile as tile
    import concourse.mybir as mybir
    from concourse import bacc

    f16, f32 = mybir.dt.float16, mybir.dt.float32
    Relu = mybir.ActivationFunctionType.Relu

    nc = bacc.Bacc("TRN2", target_bir_lowering=False, debug=False)
    # packed layouts, all [128, *], fp16 except biases/psum:
    #   xTi[p, chunk_off + ko*cw + c] = x_e[c0+c, 128*ko+p]
    #   w1i[p, g*2048 + ko*512 + fi]  = W1_e[128*ko+p, 512*g+fi]
    #   w2i[p, h*2048 + j*512 + d]    = W2_e[128*(4h+j)+p, d]
    #   bi[p, f] = b1_e[128f+p] (f<16);  bi[p, 16+d] = b2_e[128d+p]
    #   yTi[p, d*C + c] = y_e[c, 128d+p]
    xTi = nc.dram_tensor("xTi", [128, KD * C], f16, kind="ExternalInput").ap()
    w1i = nc.dram_tensor("w1i", [128, KD * F], f16, kind="ExternalInput").ap()
    w2i = nc.dram_tensor("w2i", [128, KF * D], f16, kind="ExternalInput").ap()
    bi = nc.dram_tensor("bi", [128, KF + KD], f32, kind="ExternalInput").ap()
    yTi = nc.dram_tensor("yTi", [128, KD * C], f16, kind="ExternalOutput").ap()

    (a0, a1) = chunks[0]
    cwA = a1 - a0

    with tile.TileContext(nc) as tc:
        with tc.tile_pool(name="sb", bufs=1) as sb, \
             tc.tile_pool(name="pp", bufs=1, space="PSUM") as pp:

            # --- tiles ---
            bis = sb.tile([128, KF + KD], f32, name="bis")
            w1t = sb.tile([128, KD * F], f16, name="w1t")
            w2t = sb.tile([128, KF * D], f16, name="w2t")
            xt = sb.tile([128, KD * C], f16, name="xt")
            ht = sb.tile([128, KF * C], f16, name="ht")
            ys = sb.tile([128, KD * C], f16, name="ys")
            wrm = sb.tile([128, 128], f16, name="wrm")

            p1 = [pp.tile([128, cwA], f32, name=f"p1_{fl}", tag=f"p1_{fl}",
                          bufs=1) for fl in range(4)]

            # --- PE warm-up: thin fp16 matmuls from engine-boot until real
            # data lands (~1.3us of activity; HAM ramp needs sustained busy).
            nc.vector.memset(wrm[:], 0.0)
            for _ in range(6):
                nc.tensor.matmul(p1[0][:, 0:128], wrm[:], wrm[:],
                                 start=True, stop=True)

            # --- DMA issue on sync, consumption order; first pieces small ---
            nc.sync.dma_start(bis[:], bi[:])
            nc.sync.dma_start(w1t[:, 0:512], w1i[:, 0:512])         # g0 ko0
            nc.sync.dma_start(xt[:, 0:cwA], xTi[:, 0:cwA])          # A ko0
            nc.sync.dma_start(w1t[:, 512:2048], w1i[:, 512:2048])   # g0 ko1-3
            nc.sync.dma_start(xt[:, cwA:KD * cwA], xTi[:, cwA:KD * cwA])
            nc.sync.dma_start(w1t[:, 2048:4096], w1i[:, 2048:4096])  # g1
            nc.sync.dma_start(xt[:, KD * cwA:KD * C], xTi[:, KD * cwA:KD * C])
            nc.sync.dma_start(w1t[:, 4096:6144], w1i[:, 4096:6144])  # g2
            nc.sync.dma_start(w1t[:, 6144:8192], w1i[:, 6144:8192])  # g3
            for h2 in range(G2):
                nc.sync.dma_start(w2t[:, h2 * 2048:(h2 + 1) * 2048],
                                  w2i[:, h2 * 2048:(h2 + 1) * 2048])

            # --- stage 1: h = relu(x @ W1 + b1) ---
            # wave (g, chunk): ko0 across the 4 f-tiles first (needs only the
            # earliest DMA pieces), then ko1-3 + activation per f-tile.
            for g in range(G1):
                for ci, (c0, c1) in enumerate(chunks):
                    cw = c1 - c0
                    xoff = KD * c0
                    pw = [pp.tile([128, cwA], f32, name=f"p1_{g}_{ci}_{fl}",
                                  tag=f"p1_{fl}", bufs=1) for fl in range(4)]
                    for fl in range(4):
                        nc.tensor.matmul(
                            pw[fl][:, 0:cw],
                            w1t[:, g * 2048 + fl * 128:g * 2048 + fl * 128 + 128],
                            xt[:, xoff:xoff + cw],
                            start=True, stop=False)
                    for fl in range(4):
                        f = 4 * g + fl
                        for ko in range(1, KD):
                            nc.tensor.matmul(
                                pw[fl][:, 0:cw],
                                w1t[:, g * 2048 + ko * 512 + fl * 128:
                                    g * 2048 + ko * 512 + fl * 128 + 128],
                                xt[:, xoff + ko * cw:xoff + (ko + 1) * cw],
                                start=False, stop=(ko == KD - 1))
                        nc.scalar.activation(ht[:, f * C + c0:f * C + c1],
                                             pw[fl][:, 0:cw], Relu,
                                             bias=bis[:, f:f + 1])

            # --- stage 2: y = h @ W2 + b2, d-major; output streams out ---
            for d in range(KD):
                for ci, (c0, c1) in enumerate(chunks):
                    cw = c1 - c0
                    p2 = pp.tile([128, cwA], f32, name=f"p2_{d}_{ci}",
                                 tag=f"p2_{d % 2}", bufs=2)
                    for fo in range(KF):
                        h2, j = divmod(fo, 4)
                        nc.tensor.matmul(
                            p2[:, 0:cw],
                            w2t[:, h2 * 2048 + j * 512 + d * 128:
                                h2 * 2048 + j * 512 + d * 128 + 128],
                            ht[:, fo * C + c0:fo * C + c1],
                            start=(fo == 0), stop=(fo == KF - 1))
                    nc.vector.tensor_scalar_add(ys[:, d * C + c0:d * C + c1],
                                                p2[:, 0:cw],
                                                bis[:, KF + d:KF + d + 1])
                nc.scalar.dma_start(yTi[:, d * C:(d + 1) * C],
                                    ys[:, d * C:(d + 1) * C])
    nc.compile()
    return nc


def _plan_chunks(C: int) -> list[tuple[int, int]]:
    n = max(1, -(-C // 512))
    base, rem = divmod(C, n)
    out, pos = [], 0
    for i in range(n):
        w = base + (1 if i < rem else 0)
        out.append((pos, pos + w))
        pos += w
    return out


def _get_nc(C: int):
    if C not in _cache:
        _cache[C] = _build(C, _plan_chunks(C))
    return _cache[C]


def _pack_inputs(x, W1, b1, W2, b2, idx, order, starts, C):
    chunks = _plan_chunks(C)
    in_maps, toks_per_core = [], []
    for e in range(E):
        toks = order[starts[e]:starts[e + 1]]
        toks_per_core.append(toks)
        xe = np.zeros((C, D), np.float16)
        xe[:len(toks)] = x[toks]
        xeT = xe.T  # [D, C]
        xTi = np.concatenate(
            [xeT[:, c0:c1].reshape(KD, 128, c1 - c0).transpose(1, 0, 2)
             .reshape(128, KD * (c1 - c0)) for c0, c1 in chunks], axis=1)
        w1p = np.concatenate(
            [W1[e][:, 512 * g:512 * (g + 1)].astype(np.float16)
             .reshape(KD, 128, 512)
             .transpose(1, 0, 2).reshape(128, KD * 512) for g in range(G1)], axis=1)
        w2p = np.concatenate(
            [W2[e][512 * h:512 * (h + 1), :].astype(np.float16)
             .reshape(4, 128, 512)
             .transpose(1, 0, 2).reshape(128, 4 * 512) for h in range(G2)], axis=1)
        bi = np.concatenate([b1[e].reshape(KF, 128).T,
                             b2[e].reshape(KD, 128).T], axis=1).astype(np.float32)
        in_maps.append({
            "xTi": np.ascontiguousarray(xTi),
            "w1i": np.ascontiguousarray(w1p),
            "w2i": np.ascontiguousarray(w2p),
            "bi": np.ascontiguousarray(bi),
        })
    return in_maps, toks_per_core, chunks


def kernel(x, Wg, bg, W1, b1, W2, b2):
    from concourse.bass_utils import run_bass_kernel_spmd

    x = np.asarray(x, dtype=np.float32)
    n_tok = x.shape[0]

    # host gate in f64: the mathematically-true argmax
    logits = x.astype(np.float64) @ np.asarray(Wg, np.float64) + np.asarray(bg, np.float64)
    idx = logits.argmax(1)

    counts = np.bincount(idx, minlength=E)
    order = np.argsort(idx, kind="stable")
    starts = np.zeros(E + 1, np.int64)
    starts[1:] = np.cumsum(counts)

    C = max(int(counts.max()), 256)
    C = (C + 15) // 16 * 16

    W1 = np.asarray(W1, np.float32)
    W2 = np.asarray(W2, np.float32)
    b1 = np.asarray(b1, np.float32)
    b2 = np.asarray(b2, np.float32)

    in_maps, toks_per_core, chunks = _pack_inputs(x, W1, b1, W2, b2,
                                                  idx, order, starts, C)
    nc = _get_nc(C)
    res = run_bass_kernel_spmd(nc, in_maps, core_ids=list(range(N_CORES)))

    out = np.zeros((n_tok, D), np.float32)
    for e in range(E):
        toks = toks_per_core[e]
        ye = res.results[e]["yTi"].reshape(128, KD, C).transpose(2, 1, 0) \
            .reshape(C, D).astype(np.float32)
        out[toks] = ye[:len(toks)]
    return out
